# revision 51
# baseline (speedup 1.0000x reference)
"""Trainium2 Bass kernel for SegmentationAugmentation (3D affine grid_sample, trilinear, border).

Contract: kernel(input_g, label_g, transform) -> (aug_inp f32 [8,1,128,128,128],
                                                  aug_lab bool [8,1,128,128,128])

Math (derived from the reference, with the swapaxes(-3,-1) pairs folded into
index bookkeeping; all spatial dims are 128):

  out[b,c,i,j,k] = trilinear sample of input_g[b,c,:,:,:] at positions
      p-axis (axis 2): U(i,j) = clip(64*(a00*xn(i)+a01*xn(j)+a03)+63.5, 0, 127)
      q-axis (axis 3): V(i,j) = clip(64*(a10*xn(i)+a11*xn(j)+a13)+63.5, 0, 127)
      r-axis (axis 4): W(k)   = clip(64*(a22*xn(k)+a23)+63.5, 0, 127)
  with xn(t) = (2t+1)/128 - 1 and theta = transform[:3].

This relies on the generator's z-rotation structure (theta[0:2,2]==0,
theta[2,0:2]==0), which makes U,V independent of k and W independent of (i,j).
A pure-host fallback handles arbitrary transforms.

Device pipeline (v3), data-parallel over batch (core b handles batch b).
Both volumes (input+label) ride together through every stage in fp16:

  1. ACT converts loaded f32 row-groups to fp16 (vt16).
  2. DVE z-interp in fp16 (2x mode):  Z[p,q,k] = V[..,r0(k)]*(1-fw) + V[..,r1(k)]*fw
     using run-segmented staircase slices.
  3. Z written to DRAM zpad[row, 0:128]=vol0 / [128:256]=vol1 (fp16, interleaved).
  4. Per output line i (fixed i, j=0..127): gpsimd dma_gather fetches the line's
     compact "band" (sorted set of distinct Z rows with nonzero bilinear weight,
     ~272 rows) as 512B elements (one row, both volumes).
  5. PE combines: out_line[j, k(both vols)] = sum_t S_t[i]^T @ band_t, where
     S[i] is the host-built sparse-in-dense fp16 selection matrix carrying the
     bilinear (i,j) weights. S tiles (~11.4 MiB fp16) stay resident in SBUF.
  6. ACT copies PSUM f32 -> fp16 and DMAs both output volumes.

Outputs return as fp16 and are upconverted on host; label bool threshold uses
a host fixup that recomputes voxels within FIXUP_EPS of 0.5 in the reference's
exact f32 arithmetic order.
"""
import numpy as np

N = 128
NVOX = N * N * N
NROWS = N * N           # 16384 (p,q) rows per volume
NPAD = NROWS            # no padding needed: all nonzero-weight band rows < 16384
CH = 16                 # row-slots per vt load group
NGRP = NROWS // 128 // CH  # 8 z-groups per volume
ELEM = 256              # gathered element: 1 interleaved row = 256 fp16 = 512B
FIXUP_EPS = 3e-3        # |lab-0.5| window recomputed exactly on host (max dev err ~1.3e-3)

_CACHE = {}


def _pack_idxs(idx_flat):
    """int16 dma_gather index layout: element i at [i%16, i//16], replicated to 128 partitions."""
    t = idx_flat.reshape(-1, 16).T.astype(np.int16)  # [16, n/16]
    return np.ascontiguousarray(np.tile(t, (8, 1)))  # [128, n/16]


def _host_tables(theta):
    """All transform-derived tables, computed in float64 from f32 theta."""
    th = theta.astype(np.float64)
    t = np.arange(N, dtype=np.float64)
    xn = (2.0 * t + 1.0) / N - 1.0

    U = np.clip(64.0 * (th[0, 0] * xn[:, None] + th[0, 1] * xn[None, :] + th[0, 3]) + 63.5, 0.0, 127.0)
    V = np.clip(64.0 * (th[1, 0] * xn[:, None] + th[1, 1] * xn[None, :] + th[1, 3]) + 63.5, 0.0, 127.0)
    W = np.clip(64.0 * (th[2, 2] * xn + th[2, 3]) + 63.5, 0.0, 127.0)

    p0 = np.floor(U).astype(np.int64)
    q0 = np.floor(V).astype(np.int64)
    r0 = np.floor(W).astype(np.int64)
    fu = U - p0
    fv = V - q0
    fw = (W - r0).astype(np.float32)
    r1 = np.minimum(r0 + 1, N - 1)

    w00 = (1 - fu) * (1 - fv)
    w01 = (1 - fu) * fv
    w10 = fu * (1 - fv)
    w11 = fu * fv

    # z-run decomposition: maximal segments where both r0 and r1 step by a
    # constant d in {-1,0,1}
    runs = []
    k = 0
    while k < N:
        step = 0
        if k + 1 < N:
            d = int(r0[k + 1] - r0[k])
            if d == int(r1[k + 1] - r1[k]) and d in (-1, 0, 1):
                step = d
        ln = 1
        while (k + ln < N
               and int(r0[k + ln] - r0[k]) == step * ln
               and int(r1[k + ln] - r1[k]) == step * ln):
            ln += 1
        runs.append((k, ln, int(r0[k]), int(r1[k]), step))
        k += ln

    # Per output line i: compact band of distinct Z rows with nonzero weight,
    # plus the dense fp16 selection matrix S (band-pos x j) carrying weights.
    bands = []
    s_tiles = []     # per line: [T*128, 128] fp16
    n_tiles = []
    for i in range(N):
        rows = set()
        for j in range(N):
            base = p0[i, j] * 128 + q0[i, j]
            for df, w in ((0, w00[i, j]), (1, w01[i, j]), (128, w10[i, j]), (129, w11[i, j])):
                if w != 0.0:
                    rows.add(base + df)
        band = np.array(sorted(rows), dtype=np.int64)
        assert band.max() < NROWS
        pos = {r: m for m, r in enumerate(band)}
        T = (len(band) + 127) // 128
        S = np.zeros((T * 128, 128), np.float64)
        for j in range(N):
            base = p0[i, j] * 128 + q0[i, j]
            for df, w in ((0, w00[i, j]), (1, w01[i, j]), (128, w10[i, j]), (129, w11[i, j])):
                if w != 0.0:
                    S[pos[base + df], j] += w
        bands.append(band)
        s_tiles.append(S.astype(np.float16))
        n_tiles.append(T)

    # Line processing order: descending i. The transform's a00<0 makes band
    # rows (hence the last z-group needed) ascend as i descends, so gathers
    # start while later z-groups still compute; and the regular stride lets
    # output DMAs batch 4 lines into one descriptor set.
    gmax = [int(bands[i].max() // (128 * CH)) for i in range(N)]
    gmin = [int(bands[i].min() // (128 * CH)) for i in range(N)]
    order = list(range(N - 1, -1, -1))
    if gmax[0] < gmax[N - 1]:
        order = list(range(N))  # band groups ascend with i instead

    # Batch consecutive lines per dma_gather call (amortizes the ~1us fixed
    # SWDGE launch), greedily packed under the HW ucode's 1024-index cap.
    # Bands are concatenated unpadded within a call; each line's S rows are
    # built against its call-local positions.
    IDX_CAP = 1024
    groups = []
    p0_ = 0
    while p0_ < N:
        tot = 0
        p1 = p0_
        while p1 < N and tot + len(bands[order[p1]]) <= IDX_CAP:
            tot += len(bands[order[p1]])
            p1 += 1
        groups.append(list(range(p0_, p1)))
        p0_ = p1
    if len(groups) % 2:
        # keep the call count even so the 2-buffer parity pattern repeats
        # identically across reps
        big = max(range(len(groups)), key=lambda c: len(groups[c]))
        h = len(groups[big]) // 2
        groups[big:big + 1] = [groups[big][:h], groups[big][h:]]
    ncalls = len(groups)
    call_nidx = []          # padded-to-128 index count per call
    call_bt = []            # band tiles (128-row groups) per call
    call_gneed = []         # last z-group pair needed before call can gather
    call_pos_end = []       # last processing position in the call
    pos2call = [0] * N
    idx_cat = []
    line_mm = [[] for _ in range(N)]   # per pos: list of (s_tile_packed_idx, band_tile)
    s_cat = []
    tb = 0
    for c in range(ncalls):
        poss = groups[c]
        lines = [order[p] for p in poss]
        call_pos_end.append(poss[-1])
        for p_ in poss:
            pos2call[p_] = c
        cat = np.concatenate([bands[i] for i in lines])
        nidx = len(cat)
        # pad num_idxs to a multiple of 128 (HW ucode granularity) with row 0,
        # which is always valid and lands in zero-weight positions; -1 padding
        # can emit a negative-address DMA
        nidx_pad = ((nidx + 127) // 128) * 128
        idx = np.zeros(nidx_pad, np.int64)
        idx[:nidx] = cat
        idx_cat.append(idx)
        bt = nidx_pad // 128
        call_nidx.append(nidx_pad)
        call_bt.append(bt)
        call_gneed.append(max(gmax[i] for i in lines))
        off = 0
        for p_, i in zip(poss, lines):
            b = len(bands[i])
            t0, t1 = off // 128, (off + b - 1) // 128
            Sfull = s_tiles[i]  # [T*128, 128] with rows at line-local positions
            for t in range(t0, t1 + 1):
                blk = np.zeros((128, 128), np.float16)
                # line-local position m -> call-local off+m -> tile/row
                lo = max(0, t * 128 - off)
                hi = min(b, (t + 1) * 128 - off)
                blk[(off + np.arange(lo, hi)) % 128] = Sfull[lo:hi]
                s_cat.append(blk)
                line_mm[p_].append((tb, t))
                tb += 1
            off += b
    s_packed = np.concatenate(s_cat, axis=0)          # [TOT*128, 128] fp16
    tot_tiles = tb
    idx_all = np.concatenate(idx_cat)
    call_icol = np.cumsum([0] + [n // 16 for n in call_nidx])  # idx col offset per call
    idx_packed = _pack_idxs(idx_all)                  # [128, sum(nidx)/16]

    # SP cross-rep WAR: for each z-group g, the last call whose lines' bands
    # intersect rows [2048g, 2048(g+1)).
    call_last = [0] * NGRP
    for c in range(ncalls):
        for p_ in groups[c]:
            i = order[p_]
            for g in range(gmin[i], gmax[i] + 1):
                call_last[g] = max(call_last[g], c)

    fwrep = np.stack([np.tile((1.0 - fw).astype(np.float32), (128, 1)),
                      np.tile(fw.astype(np.float32), (128, 1))]).astype(np.float32)

    return dict(runs=runs, fwrep=fwrep, s_packed=s_packed, idx_packed=idx_packed,
                order=order, line_mm=line_mm,
                line_of_pos=[order[p] for p in range(N)],
                call_nidx=call_nidx, call_bt=call_bt, call_gneed=call_gneed,
                call_icol=[int(x) for x in call_icol], call_last=call_last,
                call_pos_end=call_pos_end, pos2call=pos2call,
                ncalls=ncalls, bandt_tiles=max(call_bt), tot_tiles=tot_tiles)


def _build_program(tables, variant="full", reps=1):
    """Raw-Bass (explicit semaphore) program; see module docstring for pipeline.

    Engine streams:
      sync   (SP HWDGE):  const loads, volume loads, zpad writes
      scalar (ACT):       f32->fp16 input conversion, PSUM->fp16 output copy, out DMA
      vector (DVE):       fp16 z-interp
      gpsimd (Pool SWDGE): per-line band dma_gather
      tensor (PE):        per-line selection matmuls into PSUM
    """
    import concourse.bass as bass
    from concourse import bacc, mybir

    runs = tables["runs"]
    f32 = mybir.dt.float32
    f16 = mybir.dt.float16
    i16 = mybir.dt.int16

    BT = tables["bandt_tiles"]
    TOT = tables["tot_tiles"]
    NC = tables["ncalls"]
    IDXCOLS = tables["idx_packed"].shape[1]
    line_mm = tables["line_mm"]
    line_of_pos = tables["line_of_pos"]
    call_nidx = tables["call_nidx"]
    call_bt = tables["call_bt"]
    call_gneed = tables["call_gneed"]
    call_icol = tables["call_icol"]
    call_last = tables["call_last"]
    call_pos_end = tables["call_pos_end"]
    pos2call = tables["pos2call"]

    nc = bacc.Bacc("TRN2", target_bir_lowering=False, debug=False, num_devices=8)

    vol_in = [nc.dram_tensor(f"vol{v}", [NROWS, N], f32, kind="ExternalInput") for v in range(2)]
    sw_dram = nc.dram_tensor("sw", [TOT * 128, 128], f16, kind="ExternalInput")
    idx_dram = nc.dram_tensor("idx", [128, IDXCOLS], i16, kind="ExternalInput")
    fw_dram = nc.dram_tensor("fwrep", [2, 128, 128], f32, kind="ExternalInput")
    # single interleaved output: row (i,j) = [vol0 k0..127 | vol1 k0..127] fp16
    outi = nc.dram_tensor("outi", [NROWS, 2 * N], f16, kind="ExternalOutput")
    zpad = nc.dram_tensor("zpad", [NPAD, 2 * N], f16, kind="Internal")

    AP = bass.AP

    s_t = nc.alloc_sbuf_tensor("s_t", [128, TOT * 128], f16)
    idx_t = nc.alloc_sbuf_tensor("idx_t", [128, IDXCOLS], i16)
    fw_t = [nc.alloc_sbuf_tensor(f"fw{c}_t", [128, 128], f32) for c in range(2)]
    vt32 = [nc.alloc_sbuf_tensor(f"vt32_{s}", [128, CH * N], f32) for s in range(2)]
    # zt holds BOTH volumes of one z-group, row-interleaved like zpad
    zt = [nc.alloc_sbuf_tensor(f"zt{s}", [128, CH * 2 * N], f16) for s in range(2)]
    ztmp = [nc.alloc_sbuf_tensor(f"ztmp{s}", [128, CH * N], f16) for s in range(2)]
    bandt = [nc.alloc_sbuf_tensor(f"bandt{s}", [128, BT * ELEM], f16) for s in range(2)]
    otg = [nc.alloc_sbuf_tensor(f"otg{s}", [128, 4 * 2 * N], f16) for s in range(2)]
    ps = [nc.alloc_psum_tensor(f"ps{s}", [128, 2 * N], f32) for s in range(8)]

    NT = 16  # z-group units per rep (8 groups x 2 volumes)

    from contextlib import ExitStack
    with ExitStack() as _sctx:
        block = _sctx.enter_context(nc.Block())
        s_const = _sctx.enter_context(nc.semaphore("s_const"))
        s_vt = [_sctx.enter_context(nc.semaphore(f"s_vt{p}")) for p in range(2)]
        s_z = _sctx.enter_context(nc.semaphore("s_z"))
        s_zw = [_sctx.enter_context(nc.semaphore(f"s_zw{p}")) for p in range(2)]
        s_g = [_sctx.enter_context(nc.semaphore(f"s_g{p}")) for p in range(2)]
        s_pe = _sctx.enter_context(nc.semaphore("s_pe"))
        s_oc = _sctx.enter_context(nc.semaphore("s_oc"))
        s_o = [_sctx.enter_context(nc.semaphore(f"s_o{p}")) for p in range(2)]
        s_v = _sctx.enter_context(nc.semaphore("s_v"))
        s_ms = _sctx.enter_context(nc.semaphore("s_ms"))

        do_gather = variant in ("full", "nocomb", "noout")
        do_pe = variant in ("full", "noout")
        do_out = variant == "full"

        @block.sync
        def _(sync):
            sync.dma_start(s_t.ap(), AP(sw_dram, 0, [[128, 128], [128 * 128, TOT], [1, 128]])).then_inc(s_const, 16)
            sync.dma_start(idx_t.ap(), idx_dram.ap()).then_inc(s_const, 16)
            for c in range(2):
                sync.dma_start(fw_t[c].ap(), AP(fw_dram, c * 128 * 128, [[128, 128], [1, 128]])).then_inc(s_const, 16)
            def load(t):
                r, t_ = divmod(t, NT)
                g, v = divmod(t_, 2)
                if t >= 2:
                    sync.wait_ge(s_z, t - 1)   # vt32 WAR vs DVE reader
                sync.dma_start(
                    AP(vt32[t % 2], 0, [[CH * N, 128], [N, CH], [1, N]]),
                    AP(vol_in[v], g * CH * 128 * N, [[N, 128], [128 * N, CH], [1, N]]),
                ).then_inc(s_vt[t % 2], 16)

            def zwrite(P):
                # one write per z-group PAIR (both volumes, 512B row chunks)
                r, g = divmod(P, NGRP)
                sync.wait_ge(s_z, 2 * P + 2)
                if r >= 1 and do_gather:
                    cl = call_last[g]
                    c0 = (r - 1) * (NC // 2) + cl // 2 + 1
                    c1 = (r - 1) * (NC // 2) + (cl + 1) // 2
                    sync.wait_ge(s_g[0], 16 * c0)
                    if c1 > 0:
                        sync.wait_ge(s_g[1], 16 * c1)
                sync.dma_start(
                    AP(zpad, g * CH * 128 * 2 * N, [[2 * N, 128], [128 * 2 * N, CH], [1, 2 * N]]),
                    AP(zt[P % 2], 0, [[CH * 2 * N, 128], [2 * N, CH], [1, 2 * N]]),
                ).then_inc(s_zw[P % 2], 16)

            # keep loads ahead of each zpad write so a parked write never
            # starves the DVE of its next input group
            load(0)
            load(1)
            for P in range(reps * NGRP):
                if 2 * P + 2 < reps * NT:
                    load(2 * P + 2)
                if 2 * P + 3 < reps * NT:
                    load(2 * P + 3)
                zwrite(P)
            if variant == "full":
                for p in range(2):
                    sync.wait_ge(s_o[p], 16 * (N // 8) * reps)
            elif variant == "noout":
                sync.wait_ge(s_pe, N * reps)
            elif variant == "nocomb":
                for p in range(2):
                    sync.wait_ge(s_g[p], 16 * (NC // 2) * reps)
            else:
                for p in range(2):
                    sync.wait_ge(s_zw[p], 16 * (NGRP // 2) * reps)

        @block.scalar
        def _(scalar):
            if not do_out:
                return
            copy = mybir.ActivationFunctionType.Copy
            dstep = line_of_pos[1] - line_of_pos[0]   # +-1, regular order
            for gc in range(reps * N):
                pos = gc % N
                b = gc // 4     # 4-line output batch index
                # slot ordering keeps the batched out-DMA dst stride positive
                slot = (3 - gc % 4) if dstep < 0 else (gc % 4)
                scalar.wait_ge(s_pe, gc + 1)
                if gc % 4 == 0 and b >= 2:
                    scalar.wait_ge(s_o[b % 2], 16 * ((b - 2) // 2 + 1))  # otg WAR
                scalar.activation(
                    AP(otg[b % 2], slot * 2 * N, [[4 * 2 * N, 128], [1, 2 * N]]),
                    ps[gc % 8].ap(), copy).then_inc(s_oc, 1)
                if gc % 4 == 3:
                    scalar.wait_ge(s_oc, gc + 1)   # all 4 copies done before DMA
                    i_lo = min(line_of_pos[(gc - 3) % N], line_of_pos[gc % N])
                    scalar.dma_start(
                        AP(outi, i_lo * 128 * 2 * N, [[2 * N, 128], [128 * 2 * N, 4], [1, 2 * N]]),
                        AP(otg[b % 2], 0, [[4 * 2 * N, 128], [2 * N, 4], [1, 2 * N]]),
                    ).then_inc(s_o[b % 2], 16)

        @block.vector
        def _(vector):
            mult = mybir.AluOpType.mult
            VC = [0]

            def f0_ap(ks, ln):
                return AP(fw_t[0], ks, [[128, 128], [0, CH], [1, ln]])

            def f1_ap(ks, ln):
                return AP(fw_t[1], ks, [[128, 128], [0, CH], [1, ln]])

            def vsync(last_ins):
                last_ins.then_inc(s_v, 1)
                VC[0] += 1
                vector.wait_ge(s_v, VC[0])

            vector.wait_ge(s_const, 64)
            for t in range(reps * NT):
                P, v = divmod(t, 2)
                if t >= 1:
                    vector.wait_ge(s_z, t)   # prior group's adds drained (DVE pipeline WAR)
                if v == 0 and P >= 2:
                    vector.wait_ge(s_zw[P % 2], 16 * ((P - 2) // 2 + 1))  # zt WAR vs zpad write
                vector.wait_ge(s_vt[t % 2], 16 * (t // 2 + 1))
                s = vt32[t % 2]
                last_ins = None
                for (ks, ln, r0s, r1s, st) in runs:
                    zdst = AP(zt[P % 2], v * N + ks, [[CH * 2 * N, 128], [2 * N, CH], [1, ln]])
                    tdst = AP(ztmp[t % 2], ks, [[CH * N, 128], [N, CH], [1, ln]])
                    v0 = AP(s, r0s, [[CH * N, 128], [N, CH], [st, ln]])
                    v1 = AP(s, r1s, [[CH * N, 128], [N, CH], [st, ln]])
                    vector.tensor_tensor(zdst, v0, f0_ap(ks, ln), mult)
                    last_ins = vector.tensor_tensor(tdst, v1, f1_ap(ks, ln), mult)
                vsync(last_ins)
                for (ks, ln, r0s, r1s, st) in runs:
                    zdst = AP(zt[P % 2], v * N + ks, [[CH * 2 * N, 128], [2 * N, CH], [1, ln]])
                    tdst = AP(ztmp[t % 2], ks, [[CH * N, 128], [N, CH], [1, ln]])
                    last_ins = vector.tensor_add(zdst, zdst, tdst)
                last_ins.then_inc(s_z, 1)

        @block.gpsimd
        def _(gpsimd):
            if not do_gather:
                return
            for b in range(2):
                gpsimd.memset(bandt[b].ap(), 0.0).then_inc(s_ms, 1)
            gpsimd.wait_ge(s_ms, 2)
            nregs = {}
            for n_ in sorted(set(call_nidx)):
                nregs[n_] = gpsimd.to_reg(n_)
            gpsimd.wait_ge(s_const, 64)
            sv = AP(zpad, 0, [[ELEM, NPAD], [1, ELEM]])
            for r in range(reps):
                for c in range(NC):
                    cc = r * NC + c
                    G = call_gneed[c]
                    gpsimd.wait_ge(s_zw[0], 16 * (r * (NGRP // 2) + G // 2 + 1))
                    if G >= 1:
                        gpsimd.wait_ge(s_zw[1], 16 * (r * (NGRP // 2) + (G + 1) // 2))
                    if do_pe:
                        if cc >= 2:
                            # band buffer WAR vs PE reader: all lines of call cc-2 done
                            r2, c2 = divmod(cc - 2, NC)
                            gpsimd.wait_ge(s_pe, r2 * N + call_pos_end[c2] + 1)
                    elif cc >= 2:
                        gpsimd.wait_ge(s_g[cc % 2], 16 * (cc // 2))
                    nidx = call_nidx[c]
                    bt = call_bt[c]
                    gpsimd.dma_gather(
                        AP(bandt[cc % 2], 0, [[BT * ELEM, 128], [ELEM, bt], [1, ELEM]]),
                        sv,
                        AP(idx_t, call_icol[c], [[IDXCOLS, 128], [1, nidx // 16]]),
                        nidx, nregs[nidx], ELEM, elem_step=ELEM,
                    ).then_inc(s_g[cc % 2], 16)

        @block.tensor
        def _(tensor):
            if not do_pe:
                return
            tensor.wait_ge(s_const, 64)
            for r in range(reps):
                for pos in range(N):
                    gc = r * N + pos
                    cc = r * NC + pos2call[pos]
                    tensor.wait_ge(s_g[cc % 2], 16 * (cc // 2 + 1))
                    if do_out and gc >= 8:
                        tensor.wait_ge(s_oc, gc - 7)   # psum WAR vs ACT copy
                    mms = line_mm[pos]
                    ins = None
                    for mi, (sidx, bt_t) in enumerate(mms):
                        ins = tensor.matmul(
                            ps[gc % 8].ap(),
                            AP(s_t, sidx * 128, [[TOT * 128, 128], [1, 128]]),
                            AP(bandt[cc % 2], bt_t * ELEM, [[BT * ELEM, 128], [1, ELEM]]),
                            start=(mi == 0), stop=(mi == len(mms) - 1),
                        )
                    ins.then_inc(s_pe, 1)

    nc.compile()
    return nc


def _make_common_inputs(tables):
    return {
        "sw": np.ascontiguousarray(tables["s_packed"]),
        "idx": np.ascontiguousarray(tables["idx_packed"]),
        "fwrep": np.ascontiguousarray(tables["fwrep"]),
    }


def _exact_label_fixup(label_g, theta, lab_f, out_bool):
    """Recompute voxels of |lab_f - 0.5| < FIXUP_EPS in the reference's exact
    f32 arithmetic order (validated bit-exact against the jax reference)."""
    eps = np.float32(FIXUP_EPS)
    cand = np.abs(lab_f - np.float32(0.5)) < eps
    if not cand.any():
        return out_bool
    bb, ii, jj, kk = np.nonzero(cand.reshape(-1, N, N, N))
    v = _exact_reference_values(label_g, theta, bb, ii, jj, kk)
    out_bool.reshape(-1, N, N, N)[bb, ii, jj, kk] = v > np.float32(0.5)
    return out_bool


def _exact_reference_values(vol_g, theta, bb, ii, jj, kk):
    """Reference-order f32 trilinear values at selected voxels.

    Replicates: grid einsum (x*t0 + y*t1 + z*t2, left-assoc f32) + t3; unnorm;
    8-corner accumulation in (z,y,x) order with w=(wz*wy)*wx, out += v*w.
    """
    f32 = np.float32
    t = np.arange(N, dtype=f32)
    xn = ((f32(2.0) * t + f32(1.0)) / f32(N) - f32(1.0)).astype(f32)
    th = theta.astype(f32)

    x = xn[ii]; y = xn[jj]; z = xn[kk]

    # f32 fma via f64 (exact up to negligible double-rounding corner cases)
    def fma32(a, b, c):
        return (np.float64(a) * np.float64(b) + c.astype(np.float64)).astype(f32)

    # grid components — XLA CPU lowers the einsum as an FMA chain (verified
    # bit-exact): fma(z, t2, fma(y, t1, x*t0)) + t3
    def comp(r):
        a = fma32(y, th[r, 1], (x * th[r, 0]).astype(f32))
        a = fma32(z, th[r, 2], a)
        return (a + th[r, 3]).astype(f32)
    gx, gy, gz = comp(0), comp(1), comp(2)

    def unnorm(c):
        return np.clip(((c + f32(1.0)) * f32(N) - f32(1.0)) * f32(0.5), f32(0.0), f32(N - 1))
    ux, uy, uz = unnorm(gx), unnorm(gy), unnorm(gz)
    x0 = np.floor(ux); y0 = np.floor(uy); z0 = np.floor(uz)
    fx = (ux - x0).astype(f32); fy = (uy - y0).astype(f32); fz = (uz - z0).astype(f32)
    x0i = x0.astype(np.int64); y0i = y0.astype(np.int64); z0i = z0.astype(np.int64)
    x1i = np.minimum(x0i + 1, N - 1); y1i = np.minimum(y0i + 1, N - 1); z1i = np.minimum(z0i + 1, N - 1)

    vol = vol_g.reshape(-1, N, N, N)
    out = np.zeros(bb.shape, f32)
    one = f32(1.0)
    for zi, wz in ((z0i, (one - fz).astype(f32)), (z1i, fz)):
        for yi, wy in ((y0i, (one - fy).astype(f32)), (y1i, fy)):
            for xi, wx in ((x0i, (one - fx).astype(f32)), (x1i, fx)):
                # inp[b, c, zi, yi, xi] in transposed space == vol[b, xi, yi, zi]
                vals = vol[bb, xi, yi, zi]
                w = ((wz * wy).astype(f32) * wx).astype(f32)
                out = (out + (vals * w).astype(f32)).astype(f32)
    return out


def _host_fallback(input_g, label_g, transform):
    """Arbitrary-transform fallback: full reference computation on host."""
    bb, ii, jj, kk = np.meshgrid(np.arange(8), np.arange(N), np.arange(N), np.arange(N), indexing="ij")
    bb, ii, jj, kk = (a.reshape(-1) for a in (bb, ii, jj, kk))
    theta = transform[:3].astype(np.float32)
    aug_inp = _exact_reference_values(input_g, theta, bb, ii, jj, kk).reshape(8, 1, N, N, N)
    lab = _exact_reference_values(label_g, theta, bb, ii, jj, kk).reshape(8, 1, N, N, N)
    return aug_inp.astype(np.float32), lab > np.float32(0.5)


def kernel(input_g, label_g, transform):
    input_g = np.ascontiguousarray(input_g, dtype=np.float32)
    label_g = np.ascontiguousarray(label_g, dtype=np.float32)
    transform = np.asarray(transform, dtype=np.float32)
    theta = transform[:3]

    structured = (abs(float(theta[0, 2])) < 1e-12 and abs(float(theta[1, 2])) < 1e-12
                  and abs(float(theta[2, 0])) < 1e-12 and abs(float(theta[2, 1])) < 1e-12)
    if not structured:
        return _host_fallback(input_g, label_g, transform)

    from concourse.bass_utils import run_bass_kernel_spmd

    tables = _host_tables(theta)
    key = transform.tobytes()
    if key not in _CACHE:
        _CACHE[key] = _build_program(tables)
    nc = _CACHE[key]

    common = _make_common_inputs(tables)
    in_maps = []
    for b in range(8):
        in_maps.append(dict(common,
                            vol0=input_g[b, 0].reshape(NROWS, N),
                            vol1=label_g[b, 0].reshape(NROWS, N)))

    res = run_bass_kernel_spmd(nc, in_maps, core_ids=list(range(8)))

    aug_inp = np.empty((8, 1, N, N, N), np.float32)
    lab_f = np.empty((8, 1, N, N, N), np.float32)
    for b in range(8):
        o = res.results[b]["outi"]   # [NROWS, 256] fp16: [vol0 | vol1]
        aug_inp[b, 0] = o[:, :N].astype(np.float32).reshape(N, N, N)
        lab_f[b, 0] = o[:, N:].astype(np.float32).reshape(N, N, N)

    out_bool = lab_f > np.float32(0.5)
    out_bool = _exact_label_fixup(label_g, theta, lab_f, out_bool)
    return aug_inp, out_bool
